# revision 1
# baseline (speedup 1.0000x reference)
"""Local softmax attention (GNN message passing) on 8 Trainium2 NeuronCores.

Math (per batch b, node n):
  q/k/v = x @ W{q,k,v}.T + b{q,k,v}              [N, 128], 8 heads x d=16
  scores[n,k,h] = sum_d q[n,h,d] * k[nbr(n,k),h,d] / sqrt(d)
  attn = softmax over k (32 neighbors)
  out[n,h,d] = sum_k attn[n,k,h] * v[nbr(n,k),h,d]

Sharding: 8 cores, each owning a 2048-node range (both batches).

The dominant cost on TRN2 is the neighbor gather: SWDGE descriptor
generation costs ~8.6 ns per gathered row (Q7 software), so k|v for BOTH
batches are packed into one 1KB DRAM row per node and each gathered row
serves both batch instances -> 65536 rows per core.  dma_gather is
limited to ~1024 indices per call (the 128-deep SWDGE descriptor ring:
2048 idxs hangs the exec unit) so each 128-node tile's 4096-row gather is
split into 4 calls.

Phase 1 (projections): every core redundantly computes k|v (bf16) for
all nodes of both batches on the TensorEngine (x^T tiles stationary,
rank-1 e0-row matmul adds the bias) and writes the packed rows to a
private DRAM scratch; q (bf16) for its own nodes stays in SBUF.

Phase 2 (per 128-node tile, per batch): VectorE does q*kg (bf16 2x),
a 4-level pairwise-add tree over d for the scores (last levels fp32),
softmax denominator, attn*vg (bf16 2x, exp expanded over d by ScalarE so
both operands are dense), a 5-level pairwise tree over k, and the 1/Z
scale.  ScalarE does the exp (reading scores with a step-0 broadcast AP
so the output is already expanded over d).

SPMD: all 8 cores run the identical program; per-core variation is data
only (each core's x^T is permuted so its own 2048 nodes come first, and
gather indices are remapped into that row space).
"""

import os
import sys

sys.path.insert(0, "/opt/trn_rl_repo")

from contextlib import ExitStack

import numpy as np

import concourse.bacc as bacc
import concourse.bass as bass
import concourse.tile as tile
from concourse import mybir

HEADS = 8
P = 128
NCALL = 4          # gather calls per tile (1024 idxs each)


class Cfg:
    def __init__(self, N=16384, K=32, C=128, n_cores=8, B=2):
        self.N, self.K, self.C, self.n_cores, self.B = N, K, C, n_cores, B
        self.N_own = N // n_cores
        self.n_all_tiles = N // P
        self.n_own_tiles = self.N_own // P
        self.d = C // HEADS


def _ap(base, dims):
    return bass.AP(tensor=base.tensor, offset=base.offset,
                   ap=[base.ap[0]] + [list(x) for x in dims])


def _off(base, elems):
    return bass.AP(tensor=base.tensor, offset=base.offset + elems,
                   ap=base.ap)


def build_nc(cfg: Cfg):
    N, K, C, B = cfg.N, cfg.K, cfg.C, cfg.B
    H3 = 3 * C
    R = 2 * B * C              # packed row elems (k|v per batch): 512
    f32, bf16, i16 = mybir.dt.float32, mybir.dt.bfloat16, mybir.dt.int16
    T_all, T_own = cfg.n_all_tiles, cfg.n_own_tiles
    d = cfg.d
    ni_call = K * P // NCALL
    k_call = K // NCALL
    cols_call = ni_call // 16
    idx_cols = NCALL * cols_call

    nc = bacc.Bacc("TRN2", target_bir_lowering=False, debug=False)

    f32r = mybir.dt.float32r
    xt = nc.dram_tensor("xt", [C, B * N], f32r, kind="ExternalInput")
    wqkv = nc.dram_tensor("wqkv", [C, H3], f32r, kind="ExternalInput")
    erow = nc.dram_tensor("erow", [C, P], f32, kind="ExternalInput")
    brow = nc.dram_tensor("brow", [C, H3], f32, kind="ExternalInput")
    idxw = nc.dram_tensor("idxw", [P, T_own * idx_cols], i16,
                          kind="ExternalInput")
    outp = nc.dram_tensor("out", [B * cfg.N_own, C], f32,
                          kind="ExternalOutput")

    with tile.TileContext(nc) as tc, ExitStack() as ctx:
        const = ctx.enter_context(tc.tile_pool(name="const", bufs=1))
        xload = ctx.enter_context(tc.tile_pool(name="xload", bufs=4))
        psum = ctx.enter_context(tc.tile_pool(name="psum", bufs=4, space="PSUM"))
        kvsb = ctx.enter_context(tc.tile_pool(name="kvsb", bufs=4))
        dram = ctx.enter_context(tc.tile_pool(name="dram", bufs=1, space="DRAM"))
        gath = ctx.enter_context(tc.tile_pool(name="gath", bufs=2))
        work = ctx.enter_context(tc.tile_pool(name="work", bufs=2))
        small = ctx.enter_context(tc.tile_pool(name="small", bufs=2))

        # --- constants ---
        wqkv_sb = const.tile([C, H3], f32r)
        nc.sync.dma_start(out=wqkv_sb[:], in_=wqkv[:, :])
        del erow  # bias handled via bqrep/bvrep; erow input kept for ABI
        bvrep_sb = const.tile([P, C], f32)   # bv replicated across partitions
        b0 = brow[0:1, 2 * C : 3 * C]
        nc.gpsimd.dma_start(
            out=bvrep_sb[:],
            in_=bass.AP(tensor=b0.tensor, offset=b0.offset, ap=[[0, P], [1, C]]))
        bqrep_sb = const.tile([P, C], f32)   # bq replicated across partitions
        q0 = brow[0:1, 0:C]
        nc.gpsimd.dma_start(
            out=bqrep_sb[:],
            in_=bass.AP(tensor=q0.tensor, offset=q0.offset, ap=[[0, P], [1, C]]))
        idx_sb = const.tile([P, T_own * idx_cols], i16)
        nc.sync.dma_start(out=idx_sb[:], in_=idxw[:, :])
        q_all = const.tile([P, T_own * B * C], bf16)   # [t][b][hd]

        kv_dram = dram.tile([N, R], bf16)

        # --- phase 1: projections (both batches, all nodes) ---
        # bk cancels in the softmax (constant over the neighbor axis) and
        # bv is added exactly at the end (sum_k attn == 1), so k|v rows are
        # written WITHOUT bias and only q (own tiles) gets its bias via the
        # rank-1 e0-row matmul.
        for b in range(B):
            for t in range(T_all):
                xt_t = xload.tile([P, P], f32r)
                nc.sync.dma_start(out=xt_t[:],
                                  in_=xt[:, b * N + t * P : b * N + (t + 1) * P])
                pt = psum.tile([P, H3], f32)
                if t < T_own:
                    nc.tensor.matmul(out=pt[:], lhsT=xt_t[:],
                                     rhs=wqkv_sb[:],
                                     start=True, stop=True)
                    q_slot = (t * B + b) * C
                    nc.vector.tensor_tensor(
                        out=q_all[:, q_slot : q_slot + C], in0=pt[:, 0:C],
                        in1=bqrep_sb[:], op=mybir.AluOpType.add)
                else:
                    nc.tensor.matmul(out=pt[:, C:H3],
                                     lhsT=xt_t[:],
                                     rhs=wqkv_sb[:, C:H3],
                                     start=True, stop=True)
                kv_t = kvsb.tile([P, 2 * C], bf16)
                nc.scalar.activation(out=kv_t[:], in_=pt[:, C:H3],
                                     func=mybir.ActivationFunctionType.Copy)
                nc.sync.dma_start(
                    out=kv_dram[t * P : (t + 1) * P, 2 * b * C : 2 * (b + 1) * C],
                    in_=kv_t[:])

        # --- phase 2: gather + attention ---
        for t in range(T_own):
            kvg = gath.tile([P, K, R], bf16)
            for i in range(NCALL):
                c0 = t * idx_cols + i * cols_call
                nc.gpsimd.dma_gather(
                    out_ap=kvg[:, i * k_call : (i + 1) * k_call, :],
                    in_ap=kv_dram[:],
                    idxs_ap=idx_sb[:, c0 : c0 + cols_call],
                    num_idxs=ni_call,
                    num_idxs_reg=ni_call,
                    elem_size=R,
                )

            for b in range(B):
                kg = kvg[:, :, 2 * b * C : 2 * b * C + C]        # (k, hd)
                vg = kvg[:, :, 2 * b * C + C : 2 * (b + 1) * C]
                qt = q_all[:, (t * B + b) * C : (t * B + b + 1) * C]

                # prod[(k,h,d)] = kg * q   (bf16 2x)
                prod = work.tile([P, K * C], bf16)
                nc.vector.tensor_tensor(
                    out=_ap(prod[:], [[C, K], [1, C]]),
                    in0=kg, in1=_ap(qt, [[0, K], [1, C]]),
                    op=mybir.AluOpType.mult)

                # scores = sum_d prod : 4-level pairwise tree over d
                # L1: (k,h,8) bf16, L2: (k,h,4) bf16, L3: (k,h,2) f32,
                # L4: (k,h) f32
                st1 = small.tile([P, K * HEADS * 8], bf16)
                nc.vector.tensor_tensor(
                    out=_ap(st1[:], [[8, K * HEADS], [1, 8]]),
                    in0=_ap(prod[:], [[d, K * HEADS], [1, 8]]),
                    in1=_ap(_off(prod[:], 8), [[d, K * HEADS], [1, 8]]),
                    op=mybir.AluOpType.add)
                st2 = small.tile([P, K * HEADS * 4], bf16)
                nc.vector.tensor_tensor(
                    out=_ap(st2[:], [[4, K * HEADS], [1, 4]]),
                    in0=_ap(st1[:], [[8, K * HEADS], [1, 4]]),
                    in1=_ap(_off(st1[:], 4), [[8, K * HEADS], [1, 4]]),
                    op=mybir.AluOpType.add)
                st3 = small.tile([P, K * HEADS * 2], f32)
                nc.vector.tensor_tensor(
                    out=_ap(st3[:], [[2, K * HEADS], [1, 2]]),
                    in0=_ap(st2[:], [[4, K * HEADS], [1, 2]]),
                    in1=_ap(_off(st2[:], 2), [[4, K * HEADS], [1, 2]]),
                    op=mybir.AluOpType.add)
                scores = small.tile([P, K * HEADS], f32)       # (k, h)
                nc.vector.tensor_tensor(
                    out=_ap(scores[:], [[1, K * HEADS]]),
                    in0=_ap(st3[:], [[2, K * HEADS]]),
                    in1=_ap(_off(st3[:], 1), [[2, K * HEADS]]),
                    op=mybir.AluOpType.add)

                # expx[(k,h,d)] = exp(scores/4) expanded over d (ScalarE)
                expx = work.tile([P, K * C], bf16)
                nc.scalar.activation(
                    out=_ap(expx[:], [[C, K], [d, HEADS], [1, d]]),
                    in_=_ap(scores[:], [[HEADS, K], [1, HEADS], [0, d]]),
                    func=mybir.ActivationFunctionType.Exp,
                    scale=1.0 / float(np.sqrt(d)))

                # z[h] = sum_k expx[k,h,0]
                z = small.tile([P, HEADS], f32)
                nc.vector.tensor_reduce(
                    out=z[:],
                    in_=_ap(expx[:], [[d, HEADS], [C, K]]),
                    axis=mybir.AxisListType.X, op=mybir.AluOpType.add)
                rz = small.tile([P, HEADS], f32)
                nc.vector.reciprocal(rz[:], z[:])

                # prod2 = expx * vg  (bf16 2x, both dense)
                prod2 = work.tile([P, K * C], bf16)
                nc.vector.tensor_tensor(
                    out=_ap(prod2[:], [[C, K], [1, C]]),
                    in0=vg, in1=_ap(expx[:], [[C, K], [1, C]]),
                    op=mybir.AluOpType.mult)

                # acc = sum_k prod2 : 5-level pairwise tree over k
                at1 = small.tile([P, K * C // 2], bf16)
                nc.vector.tensor_tensor(
                    out=at1[:], in0=prod2[:, 0 : K * C // 2],
                    in1=prod2[:, K * C // 2 : K * C],
                    op=mybir.AluOpType.add)
                at2 = small.tile([P, K * C // 4], bf16)
                nc.vector.tensor_tensor(
                    out=at2[:], in0=at1[:, 0 : K * C // 4],
                    in1=at1[:, K * C // 4 : K * C // 2],
                    op=mybir.AluOpType.add)
                at3 = small.tile([P, K * C // 8], bf16)
                nc.vector.tensor_tensor(
                    out=at3[:], in0=at2[:, 0 : K * C // 8],
                    in1=at2[:, K * C // 8 : K * C // 4],
                    op=mybir.AluOpType.add)
                at4 = small.tile([P, 2 * C], bf16)
                nc.vector.tensor_tensor(
                    out=at4[:], in0=at3[:, 0 : 2 * C], in1=at3[:, 2 * C : 4 * C],
                    op=mybir.AluOpType.add)
                acc = small.tile([P, C], f32)
                nc.vector.tensor_tensor(
                    out=acc[:], in0=at4[:, 0:C], in1=at4[:, C : 2 * C],
                    op=mybir.AluOpType.add)

                # out = acc * (1/z) + bv   (bv exact: sum_k attn == 1)
                sc = small.tile([P, C], f32)
                nc.vector.tensor_tensor(
                    out=sc[:], in0=acc[:],
                    in1=_ap(rz[:], [[1, HEADS], [0, d]]),
                    op=mybir.AluOpType.mult)
                outt = small.tile([P, C], f32)
                nc.vector.tensor_tensor(
                    out=outt[:], in0=sc[:], in1=bvrep_sb[:],
                    op=mybir.AluOpType.add)
                nc.sync.dma_start(
                    out=outp[b * cfg.N_own + t * P : b * cfg.N_own + (t + 1) * P, :],
                    in_=outt[:])

    nc.compile()
    return nc


def make_in_maps(cfg: Cfg, x, Wq, bq, Wk, bk, Wv, bv, neighbor_index):
    N, K, C, B = cfg.N, cfg.K, cfg.C, cfg.B
    T_own, N_own = cfg.n_own_tiles, cfg.N_own

    x = np.asarray(x, np.float32)
    wqkv = np.ascontiguousarray(np.concatenate(
        [np.asarray(Wq, np.float32).T, np.asarray(Wk, np.float32).T,
         np.asarray(Wv, np.float32).T], axis=1))
    erow = np.zeros((C, P), np.float32)
    erow[0, :] = 1.0
    brow = np.zeros((C, 3 * C), np.float32)
    brow[0, :] = np.concatenate(
        [np.asarray(bq, np.float32), np.asarray(bk, np.float32),
         np.asarray(bv, np.float32)])
    nbr = np.asarray(neighbor_index, np.int64)
    xtb = np.ascontiguousarray(x.transpose(0, 2, 1))   # [B, C, N]

    in_maps = []
    for c in range(cfg.n_cores):
        own = np.arange(c * N_own, (c + 1) * N_own)
        rest = np.concatenate(
            [np.arange(0, c * N_own), np.arange((c + 1) * N_own, N)])
        perm = np.concatenate([own, rest])
        inv = np.empty(N, np.int64)
        inv[perm] = np.arange(N)

        xt_c = np.ascontiguousarray(
            xtb[:, :, perm].transpose(1, 0, 2).reshape(C, B * N))

        nb = inv[nbr[own]]                                  # [N_own, K]
        vals = nb.reshape(T_own, P, K).transpose(0, 2, 1)   # [T, k, nl]
        vals = vals.reshape(T_own, NCALL, (K // NCALL) * P)
        a = vals.reshape(T_own, NCALL, (K // NCALL) * P // 16, 16)
        a = a.transpose(3, 0, 1, 2)                          # [16, T, NCALL, S]
        rep = np.tile(a, (8, 1, 1, 1))                       # [128, ...]
        idxw = np.ascontiguousarray(
            rep.reshape(P, T_own * (K * P // 16)).astype(np.int16))

        in_maps.append({
            "xt": xt_c, "wqkv": wqkv, "erow": erow, "brow": brow,
            "idxw": idxw,
        })
    return in_maps


_CACHE = {}


def _get_nc(cfg: Cfg):
    key = (cfg.N, cfg.K, cfg.C, cfg.n_cores, cfg.B)
    if key not in _CACHE:
        _CACHE[key] = build_nc(cfg)
    return _CACHE[key]


def kernel(x, Wq, bq, Wk, bk, Wv, bv, neighbor_index, _trace=False):
    from concourse.bass_utils import run_bass_kernel_spmd

    x = np.asarray(x)
    B, N, C = x.shape
    K = np.asarray(neighbor_index).shape[1]
    cfg = Cfg(N=N, K=K, C=C, n_cores=8, B=B)
    nc = _get_nc(cfg)
    in_maps = make_in_maps(cfg, x, Wq, bq, Wk, bk, Wv, bv, neighbor_index)
    res = run_bass_kernel_spmd(nc, in_maps, core_ids=list(range(cfg.n_cores)),
                               trace=_trace)
    out = np.empty((B, N, C), np.float32)
    for c in range(cfg.n_cores):
        o = res.results[c]["out"].reshape(B, cfg.N_own, C)
        out[:, c * cfg.N_own : (c + 1) * cfg.N_own, :] = o
    if _trace:
        kernel.last_results = res
    return out



# revision 2
# speedup vs baseline: 1.2998x; 1.2998x over previous
"""Local softmax attention (GNN message passing) on 8 Trainium2 NeuronCores.

Math (per batch b, node n):
  q/k/v = x @ W{q,k,v}.T + b{q,k,v}              [N, 128], 8 heads x d=16
  scores[n,k,h] = sum_d q[n,h,d] * k[nbr(n,k),h,d] / sqrt(d)
  attn = softmax over k (32 neighbors)
  out[n,h,d] = sum_k attn[n,k,h] * v[nbr(n,k),h,d]

Sharding: 8 cores, each owning a 2048-node range (both batches).

The dominant cost on TRN2 is the neighbor gather: SWDGE descriptor
generation costs ~8.6 ns per gathered row (Q7 software), so k|v for BOTH
batches are packed into one 1KB DRAM row per node and each gathered row
serves both batch instances -> 65536 rows per core.  dma_gather is
limited to ~1024 indices per call (the 128-deep SWDGE descriptor ring:
2048 idxs hangs the exec unit) so each 128-node tile's 4096-row gather is
split into 4 calls.

Phase 1 (projections): every core redundantly computes k|v (bf16) for
all nodes of both batches on the TensorEngine (x^T tiles stationary,
all-bf16 matmul) and writes the packed rows to a private DRAM scratch in
1 MiB group transfers (8 tiles x both batches per store); q (bf16) for
its own nodes stays in SBUF with the bq bias added on VectorE.  bk
cancels in the softmax; bv is added exactly at the end (sum_k attn == 1).

Phase 2 (per 128-node tile, per batch): VectorE does q*kg (bf16 2x),
a 4-level pairwise-add tree over d for the scores (last levels fp32),
softmax denominator, attn*vg (bf16 2x, exp expanded over d by ScalarE so
both operands are dense), a 5-level pairwise tree over k, the 1/Z
scale and the bv add.  ScalarE does the exp (reading scores with a
step-0 broadcast AP so the output is already expanded over d).

SPMD: all 8 cores run the identical program; per-core variation is data
only (each core's x^T is permuted so its own 2048 nodes come first, and
gather indices are remapped into that row space).
"""

import os
import sys

sys.path.insert(0, "/opt/trn_rl_repo")

from contextlib import ExitStack

import numpy as np

import concourse.bacc as bacc
import concourse.bass as bass
import concourse.tile as tile
from concourse import mybir

HEADS = 8
P = 128
NCALL = 4          # gather calls per tile (1024 idxs each)
GT = 8             # phase-1 tiles per DMA group


class Cfg:
    def __init__(self, N=16384, K=32, C=128, n_cores=8, B=2):
        self.N, self.K, self.C, self.n_cores, self.B = N, K, C, n_cores, B
        self.N_own = N // n_cores
        self.n_all_tiles = N // P
        self.n_own_tiles = self.N_own // P
        self.d = C // HEADS


def _ap(base, dims):
    return bass.AP(tensor=base.tensor, offset=base.offset,
                   ap=[base.ap[0]] + [list(x) for x in dims])


def _off(base, elems):
    return bass.AP(tensor=base.tensor, offset=base.offset + elems,
                   ap=base.ap)


def build_nc(cfg: Cfg):
    N, K, C, B = cfg.N, cfg.K, cfg.C, cfg.B
    H3 = 3 * C
    R = 2 * B * C              # packed row elems (k|v per batch): 512
    f32, bf16, i16 = mybir.dt.float32, mybir.dt.bfloat16, mybir.dt.int16
    T_all, T_own = cfg.n_all_tiles, cfg.n_own_tiles
    d = cfg.d
    ni_call = K * P // NCALL
    k_call = K // NCALL
    cols_call = ni_call // 16
    idx_cols = NCALL * cols_call
    n_grp = T_all // GT

    nc = bacc.Bacc("TRN2", target_bir_lowering=False, debug=False)

    xt = nc.dram_tensor("xt", [C, B * N], bf16, kind="ExternalInput")
    wqkv = nc.dram_tensor("wqkv", [C, H3], bf16, kind="ExternalInput")
    brow = nc.dram_tensor("brow", [C, H3], f32, kind="ExternalInput")
    idxw = nc.dram_tensor("idxw", [P, T_own * idx_cols], i16,
                          kind="ExternalInput")
    outp = nc.dram_tensor("out", [B * cfg.N_own, C], f32,
                          kind="ExternalOutput")

    with tile.TileContext(nc) as tc, ExitStack() as ctx:
        const = ctx.enter_context(tc.tile_pool(name="const", bufs=1))
        xload = ctx.enter_context(tc.tile_pool(name="xload", bufs=2))
        psum = ctx.enter_context(tc.tile_pool(name="psum", bufs=4, space="PSUM"))
        kvgrp = ctx.enter_context(tc.tile_pool(name="kvgrp", bufs=2))
        dram = ctx.enter_context(tc.tile_pool(name="dram", bufs=1, space="DRAM"))
        gath = ctx.enter_context(tc.tile_pool(name="gath", bufs=2))
        worka = ctx.enter_context(tc.tile_pool(name="worka", bufs=2))
        workb = ctx.enter_context(tc.tile_pool(name="workb", bufs=2))
        small = ctx.enter_context(tc.tile_pool(name="small", bufs=2))

        # --- constants ---
        wqkv_sb = const.tile([C, H3], bf16)
        nc.sync.dma_start(out=wqkv_sb[:], in_=wqkv[:, :])
        bvrep_sb = const.tile([P, C], f32)   # bv replicated across partitions
        b0 = brow[0:1, 2 * C : 3 * C]
        nc.gpsimd.dma_start(
            out=bvrep_sb[:],
            in_=bass.AP(tensor=b0.tensor, offset=b0.offset, ap=[[0, P], [1, C]]))
        bqrep_sb = const.tile([P, C], f32)   # bq replicated across partitions
        q0 = brow[0:1, 0:C]
        nc.gpsimd.dma_start(
            out=bqrep_sb[:],
            in_=bass.AP(tensor=q0.tensor, offset=q0.offset, ap=[[0, P], [1, C]]))
        idx_sb = const.tile([P, T_own * idx_cols], i16)
        nc.sync.dma_start(out=idx_sb[:], in_=idxw[:, :])
        q_all = const.tile([P, T_own * B * C], bf16)   # [t][b][hd]

        kv_dram = dram.tile([N, R], bf16)

        # --- phase 1: projections (both batches, all nodes) ---
        # Grouped DMA: load 8 tiles of x^T per batch in one 256 KiB
        # transfer, store 8 tiles' packed k|v rows (both batches) in one
        # 1 MiB transfer.
        for g in range(n_grp):
            xt_g = xload.tile([P, B, GT, P], bf16)
            for b in range(B):
                nc.sync.dma_start(
                    out=xt_g[:, b],
                    in_=xt[:, b * N + g * GT * P : b * N + (g + 1) * GT * P])
            kv_g = kvgrp.tile([P, GT, B, 2 * C], bf16)
            for t8 in range(GT):
                t = g * GT + t8
                for b in range(B):
                    pt = psum.tile([P, H3], f32)
                    if t < T_own:
                        nc.tensor.matmul(out=pt[:], lhsT=xt_g[:, b, t8],
                                         rhs=wqkv_sb[:],
                                         start=True, stop=True)
                        q_slot = (t * B + b) * C
                        nc.vector.tensor_tensor(
                            out=q_all[:, q_slot : q_slot + C], in0=pt[:, 0:C],
                            in1=bqrep_sb[:], op=mybir.AluOpType.add)
                    else:
                        nc.tensor.matmul(out=pt[:, C:H3],
                                         lhsT=xt_g[:, b, t8],
                                         rhs=wqkv_sb[:, C:H3],
                                         start=True, stop=True)
                    nc.scalar.activation(out=kv_g[:, t8, b], in_=pt[:, C:H3],
                                         func=mybir.ActivationFunctionType.Copy)
            # rows g*1024 + t8*128 + p, all 512 cols
            dst = kv_dram[g * GT * P : (g + 1) * GT * P, :]
            nc.sync.dma_start(
                out=bass.AP(tensor=dst.tensor, offset=dst.offset,
                            ap=[[R, P], [R * P, GT], [1, R]]),
                in_=kv_g[:])

        # --- phase 2: gather + attention ---
        for t in range(T_own):
            kvg = gath.tile([P, K, R], bf16)
            for i in range(NCALL):
                c0 = t * idx_cols + i * cols_call
                nc.gpsimd.dma_gather(
                    out_ap=kvg[:, i * k_call : (i + 1) * k_call, :],
                    in_ap=kv_dram[:],
                    idxs_ap=idx_sb[:, c0 : c0 + cols_call],
                    num_idxs=ni_call,
                    num_idxs_reg=ni_call,
                    elem_size=R,
                )

            for b in range(B):
                kg = kvg[:, :, 2 * b * C : 2 * b * C + C]        # (k, hd)
                vg = kvg[:, :, 2 * b * C + C : 2 * (b + 1) * C]
                qt = q_all[:, (t * B + b) * C : (t * B + b + 1) * C]

                # prod[(k,h,d)] = kg * q   (bf16 2x)
                prod = worka.tile([P, K * C], bf16)
                nc.vector.tensor_tensor(
                    out=_ap(prod[:], [[C, K], [1, C]]),
                    in0=kg, in1=_ap(qt, [[0, K], [1, C]]),
                    op=mybir.AluOpType.mult)

                # scores = sum_d prod : 4-level pairwise tree over d
                st1 = small.tile([P, K * HEADS * 8], bf16)
                nc.vector.tensor_tensor(
                    out=_ap(st1[:], [[8, K * HEADS], [1, 8]]),
                    in0=_ap(prod[:], [[d, K * HEADS], [1, 8]]),
                    in1=_ap(_off(prod[:], 8), [[d, K * HEADS], [1, 8]]),
                    op=mybir.AluOpType.add)
                st2 = small.tile([P, K * HEADS * 4], bf16)
                nc.vector.tensor_tensor(
                    out=_ap(st2[:], [[4, K * HEADS], [1, 4]]),
                    in0=_ap(st1[:], [[8, K * HEADS], [1, 4]]),
                    in1=_ap(_off(st1[:], 4), [[8, K * HEADS], [1, 4]]),
                    op=mybir.AluOpType.add)
                st3 = small.tile([P, K * HEADS * 2], f32)
                nc.vector.tensor_tensor(
                    out=_ap(st3[:], [[2, K * HEADS], [1, 2]]),
                    in0=_ap(st2[:], [[4, K * HEADS], [1, 2]]),
                    in1=_ap(_off(st2[:], 2), [[4, K * HEADS], [1, 2]]),
                    op=mybir.AluOpType.add)
                scores = small.tile([P, K * HEADS], f32)       # (k, h)
                nc.vector.tensor_tensor(
                    out=_ap(scores[:], [[1, K * HEADS]]),
                    in0=_ap(st3[:], [[2, K * HEADS]]),
                    in1=_ap(_off(st3[:], 1), [[2, K * HEADS]]),
                    op=mybir.AluOpType.add)

                # expx[(k,h,d)] = exp(scores/4) expanded over d (ScalarE)
                expx = workb.tile([P, K * C], bf16)
                nc.scalar.activation(
                    out=_ap(expx[:], [[C, K], [d, HEADS], [1, d]]),
                    in_=_ap(scores[:], [[HEADS, K], [1, HEADS], [0, d]]),
                    func=mybir.ActivationFunctionType.Exp,
                    scale=1.0 / float(np.sqrt(d)))

                # z[h] = sum_k expx[k,h,0]
                z = small.tile([P, HEADS], f32)
                nc.vector.tensor_reduce(
                    out=z[:],
                    in_=_ap(expx[:], [[d, HEADS], [C, K]]),
                    axis=mybir.AxisListType.X, op=mybir.AluOpType.add)
                rz = small.tile([P, HEADS], f32)
                nc.vector.reciprocal(rz[:], z[:])

                # prod2 = expx * vg  (bf16 2x, both dense)
                prod2 = worka.tile([P, K * C], bf16)
                nc.vector.tensor_tensor(
                    out=_ap(prod2[:], [[C, K], [1, C]]),
                    in0=vg, in1=_ap(expx[:], [[C, K], [1, C]]),
                    op=mybir.AluOpType.mult)

                # acc = sum_k prod2 : 5-level pairwise tree over k
                at1 = small.tile([P, K * C // 2], bf16)
                nc.vector.tensor_tensor(
                    out=at1[:], in0=prod2[:, 0 : K * C // 2],
                    in1=prod2[:, K * C // 2 : K * C],
                    op=mybir.AluOpType.add)
                at2 = small.tile([P, K * C // 4], bf16)
                nc.vector.tensor_tensor(
                    out=at2[:], in0=at1[:, 0 : K * C // 4],
                    in1=at1[:, K * C // 4 : K * C // 2],
                    op=mybir.AluOpType.add)
                at3 = small.tile([P, K * C // 8], bf16)
                nc.vector.tensor_tensor(
                    out=at3[:], in0=at2[:, 0 : K * C // 8],
                    in1=at2[:, K * C // 8 : K * C // 4],
                    op=mybir.AluOpType.add)
                at4 = small.tile([P, 2 * C], bf16)
                nc.vector.tensor_tensor(
                    out=at4[:], in0=at3[:, 0 : 2 * C], in1=at3[:, 2 * C : 4 * C],
                    op=mybir.AluOpType.add)
                acc = small.tile([P, C], f32)
                nc.vector.tensor_tensor(
                    out=acc[:], in0=at4[:, 0:C], in1=at4[:, C : 2 * C],
                    op=mybir.AluOpType.add)

                # out = acc * (1/z) + bv   (bv exact: sum_k attn == 1)
                sc = small.tile([P, C], f32)
                nc.vector.tensor_tensor(
                    out=sc[:], in0=acc[:],
                    in1=_ap(rz[:], [[1, HEADS], [0, d]]),
                    op=mybir.AluOpType.mult)
                outt = small.tile([P, C], f32)
                nc.vector.tensor_tensor(
                    out=outt[:], in0=sc[:], in1=bvrep_sb[:],
                    op=mybir.AluOpType.add)
                nc.sync.dma_start(
                    out=outp[b * cfg.N_own + t * P : b * cfg.N_own + (t + 1) * P, :],
                    in_=outt[:])

    nc.compile()
    return nc


def make_in_maps(cfg: Cfg, x, Wq, bq, Wk, bk, Wv, bv, neighbor_index):
    import ml_dtypes

    N, K, C, B = cfg.N, cfg.K, cfg.C, cfg.B
    T_own, N_own = cfg.n_own_tiles, cfg.N_own
    bf16 = ml_dtypes.bfloat16

    x = np.asarray(x, np.float32)
    wqkv = np.ascontiguousarray(np.concatenate(
        [np.asarray(Wq, np.float32).T, np.asarray(Wk, np.float32).T,
         np.asarray(Wv, np.float32).T], axis=1)).astype(bf16)
    brow = np.zeros((C, 3 * C), np.float32)
    brow[0, :] = np.concatenate(
        [np.asarray(bq, np.float32), np.asarray(bk, np.float32),
         np.asarray(bv, np.float32)])
    nbr = np.asarray(neighbor_index, np.int64)
    xtb = np.ascontiguousarray(x.transpose(0, 2, 1))   # [B, C, N]

    in_maps = []
    for c in range(cfg.n_cores):
        own = np.arange(c * N_own, (c + 1) * N_own)
        rest = np.concatenate(
            [np.arange(0, c * N_own), np.arange((c + 1) * N_own, N)])
        perm = np.concatenate([own, rest])
        inv = np.empty(N, np.int64)
        inv[perm] = np.arange(N)

        xt_c = np.ascontiguousarray(
            xtb[:, :, perm].transpose(1, 0, 2).reshape(C, B * N)).astype(bf16)

        nb = inv[nbr[own]]                                  # [N_own, K]
        vals = nb.reshape(T_own, P, K).transpose(0, 2, 1)   # [T, k, nl]
        vals = vals.reshape(T_own, NCALL, (K // NCALL) * P)
        a = vals.reshape(T_own, NCALL, (K // NCALL) * P // 16, 16)
        a = a.transpose(3, 0, 1, 2)                          # [16, T, NCALL, S]
        rep = np.tile(a, (8, 1, 1, 1))                       # [128, ...]
        idxw = np.ascontiguousarray(
            rep.reshape(P, T_own * (K * P // 16)).astype(np.int16))

        in_maps.append({
            "xt": xt_c, "wqkv": wqkv, "brow": brow, "idxw": idxw,
        })
    return in_maps


_CACHE = {}


def _get_nc(cfg: Cfg):
    key = (cfg.N, cfg.K, cfg.C, cfg.n_cores, cfg.B)
    if key not in _CACHE:
        _CACHE[key] = build_nc(cfg)
    return _CACHE[key]


def kernel(x, Wq, bq, Wk, bk, Wv, bv, neighbor_index, _trace=False):
    from concourse.bass_utils import run_bass_kernel_spmd

    x = np.asarray(x)
    B, N, C = x.shape
    K = np.asarray(neighbor_index).shape[1]
    cfg = Cfg(N=N, K=K, C=C, n_cores=8, B=B)
    nc = _get_nc(cfg)
    in_maps = make_in_maps(cfg, x, Wq, bq, Wk, bk, Wv, bv, neighbor_index)
    res = run_bass_kernel_spmd(nc, in_maps, core_ids=list(range(cfg.n_cores)),
                               trace=_trace)
    out = np.empty((B, N, C), np.float32)
    for c in range(cfg.n_cores):
        o = res.results[c]["out"].reshape(B, cfg.N_own, C)
        out[:, c * cfg.N_own : (c + 1) * cfg.N_own, :] = o
    if _trace:
        kernel.last_results = res
    return out


# revision 7
# speedup vs baseline: 1.4086x; 1.0837x over previous
"""Local softmax attention (GNN message passing) on 8 Trainium2 NeuronCores.

Math (per batch b, node n):
  q/k/v = x @ W{q,k,v}.T + b{q,k,v}              [N, 128], 8 heads x d=16
  scores[n,k,h] = sum_d q[n,h,d] * k[nbr(n,k),h,d] / sqrt(d)
  attn = softmax over k (32 neighbors)
  out[n,h,d] = sum_k attn[n,k,h] * v[nbr(n,k),h,d]

Sharding: 8 cores, each owning a 2048-node range (both batches).

The dominant cost on TRN2 is the neighbor gather: SWDGE descriptor
generation costs ~8.6 ns per gathered row (Q7 software), so k|v for BOTH
batches are packed into one 1KB DRAM row per node and each gathered row
serves both batch instances -> 65536 rows per core.  dma_gather is
limited to ~1024 indices per call (the 128-deep SWDGE descriptor ring:
2048 idxs hangs the exec unit) so each 128-node tile's 4096-row gather is
split into 4 calls.

Phase 1 (projections): every core redundantly computes k|v (bf16) for
all nodes of both batches on the TensorEngine (x^T tiles stationary,
all-bf16 matmul) and writes the packed rows to a private DRAM scratch in
1 MiB group transfers (8 tiles x both batches per store); q (bf16) for
its own nodes stays in SBUF with the bq bias added on VectorE.  bk
cancels in the softmax; bv is added exactly at the end (sum_k attn == 1).

Phase 2 (per 128-node tile, per batch): VectorE does q*kg (bf16 2x),
a 4-level pairwise-add tree over d for the scores (last levels fp32),
softmax denominator, attn*vg (bf16 2x, exp expanded over d by ScalarE so
both operands are dense), a 5-level pairwise tree over k, the 1/Z
scale and the bv add.  ScalarE does the exp (reading scores with a
step-0 broadcast AP so the output is already expanded over d).

SPMD: all 8 cores run the identical program; per-core variation is data
only (each core's x^T is permuted so its own 2048 nodes come first, and
gather indices are remapped into that row space).
"""

import os
import sys

sys.path.insert(0, "/opt/trn_rl_repo")

from contextlib import ExitStack

import numpy as np

import concourse.bacc as bacc
import concourse.bass as bass
import concourse.tile as tile
from concourse import mybir

HEADS = 8
P = 128
NCALL = 4          # gather calls per tile (1024 idxs each)
GT = 8             # phase-1 tiles per DMA group


class Cfg:
    def __init__(self, N=16384, K=32, C=128, n_cores=8, B=2):
        self.N, self.K, self.C, self.n_cores, self.B = N, K, C, n_cores, B
        self.N_own = N // n_cores
        self.n_all_tiles = N // P
        self.n_own_tiles = self.N_own // P
        self.d = C // HEADS


def _ap(base, dims):
    return bass.AP(tensor=base.tensor, offset=base.offset,
                   ap=[base.ap[0]] + [list(x) for x in dims])


def _off(base, elems):
    return bass.AP(tensor=base.tensor, offset=base.offset + elems,
                   ap=base.ap)


def build_nc(cfg: Cfg):
    N, K, C, B = cfg.N, cfg.K, cfg.C, cfg.B
    H3 = 3 * C
    R = 2 * B * C              # packed row elems (k|v per batch): 512
    f32, bf16, i16 = mybir.dt.float32, mybir.dt.bfloat16, mybir.dt.int16
    T_all, T_own = cfg.n_all_tiles, cfg.n_own_tiles
    d = cfg.d
    ni_call = K * P // NCALL
    k_call = K // NCALL
    cols_call = ni_call // 16
    idx_cols = NCALL * cols_call
    n_grp = T_all // GT

    nc = bacc.Bacc("TRN2", target_bir_lowering=False, debug=False)

    xt = nc.dram_tensor("xt", [C, B * N], bf16, kind="ExternalInput")
    wqkv = nc.dram_tensor("wqkv", [C, H3], bf16, kind="ExternalInput")
    brow = nc.dram_tensor("brow", [C, H3], f32, kind="ExternalInput")
    idxw = nc.dram_tensor("idxw", [P, T_own * idx_cols], i16,
                          kind="ExternalInput")
    outp = nc.dram_tensor("out", [B * cfg.N_own, C], f32,
                          kind="ExternalOutput")

    with tile.TileContext(nc) as tc, ExitStack() as ctx:
        const = ctx.enter_context(tc.tile_pool(name="const", bufs=1))
        xload = ctx.enter_context(tc.tile_pool(name="xload", bufs=2))
        psum = ctx.enter_context(tc.tile_pool(name="psum", bufs=4, space="PSUM"))
        kvgrp = ctx.enter_context(tc.tile_pool(name="kvgrp", bufs=2))
        dram = ctx.enter_context(tc.tile_pool(name="dram", bufs=1, space="DRAM"))
        gath = ctx.enter_context(tc.tile_pool(name="gath", bufs=2 * NCALL))
        worka = ctx.enter_context(tc.tile_pool(name="worka", bufs=2))
        workb = ctx.enter_context(tc.tile_pool(name="workb", bufs=1))
        small = ctx.enter_context(tc.tile_pool(name="small", bufs=2))

        # --- constants ---
        wqkv_sb = const.tile([C, H3], bf16)
        nc.sync.dma_start(out=wqkv_sb[:], in_=wqkv[:, :])
        bvrep_sb = const.tile([P, C], f32)   # bv replicated across partitions
        b0 = brow[0:1, 2 * C : 3 * C]
        nc.gpsimd.dma_start(
            out=bvrep_sb[:],
            in_=bass.AP(tensor=b0.tensor, offset=b0.offset, ap=[[0, P], [1, C]]))
        bqrep_sb = const.tile([P, C], f32)   # bq replicated across partitions
        q0 = brow[0:1, 0:C]
        nc.gpsimd.dma_start(
            out=bqrep_sb[:],
            in_=bass.AP(tensor=q0.tensor, offset=q0.offset, ap=[[0, P], [1, C]]))
        idx_sb = const.tile([P, T_own * idx_cols], i16)
        nc.sync.dma_start(out=idx_sb[:], in_=idxw[:, :])
        q_all = const.tile([P, T_own * B * C], bf16)   # [t][b][hd]

        kv_dram = dram.tile([N, R], bf16)

        # --- phase 1: projections (both batches, all nodes) ---
        # Grouped DMA: load 8 tiles of x^T per batch in one 256 KiB
        # transfer, store 8 tiles' packed k|v rows (both batches) in one
        # 1 MiB transfer.
        for g in range(n_grp):
            xt_g = xload.tile([P, B, GT, P], bf16)
            for b in range(B):
                nc.sync.dma_start(
                    out=xt_g[:, b],
                    in_=xt[:, b * N + g * GT * P : b * N + (g + 1) * GT * P])
            kv_g = kvgrp.tile([P, GT, B, 2 * C], bf16)
            for t8 in range(GT):
                t = g * GT + t8
                for b in range(B):
                    pt = psum.tile([P, H3], f32)
                    if t < T_own:
                        nc.tensor.matmul(out=pt[:], lhsT=xt_g[:, b, t8],
                                         rhs=wqkv_sb[:],
                                         start=True, stop=True)
                        q_slot = (t * B + b) * C
                        nc.vector.tensor_tensor(
                            out=q_all[:, q_slot : q_slot + C], in0=pt[:, 0:C],
                            in1=bqrep_sb[:], op=mybir.AluOpType.add)
                    else:
                        nc.tensor.matmul(out=pt[:, C:H3],
                                         lhsT=xt_g[:, b, t8],
                                         rhs=wqkv_sb[:, C:H3],
                                         start=True, stop=True)
                    # split PSUM->SBUF copies between ScalarE and VectorE
                    if b == 0:
                        nc.scalar.activation(
                            out=kv_g[:, t8, b], in_=pt[:, C:H3],
                            func=mybir.ActivationFunctionType.Copy)
                    else:
                        nc.vector.tensor_scalar_add(kv_g[:, t8, b],
                                                    pt[:, C:H3], 0.0)
            # rows g*1024 + t8*128 + p, all 512 cols
            dst = kv_dram[g * GT * P : (g + 1) * GT * P, :]
            nc.sync.dma_start(
                out=bass.AP(tensor=dst.tensor, offset=dst.offset,
                            ap=[[R, P], [R * P, GT], [1, R]]),
                in_=kv_g[:])

        # --- phase 2: gather + attention ---
        # One register for all gather calls (a per-call MOVE creates WAR
        # hazards that stall the Pool queue); one SBUF sub-tile per call so
        # consecutive gathers don't WAW-serialize on transfer completion.
        ni_reg = nc.gpsimd.to_reg(ni_call)
        for t in range(T_own):
            kvgs = []
            for i in range(NCALL):
                kvg_i = gath.tile([P, k_call, R], bf16)
                c0 = t * idx_cols + i * cols_call
                nc.gpsimd.dma_gather(
                    out_ap=kvg_i[:],
                    in_ap=kv_dram[:],
                    idxs_ap=idx_sb[:, c0 : c0 + cols_call],
                    num_idxs=ni_call,
                    num_idxs_reg=ni_reg,
                    elem_size=R,
                )
                kvgs.append(kvg_i)

            for b in range(B):
                qt = q_all[:, (t * B + b) * C : (t * B + b + 1) * C]

                # prod[(k,h,d)] = kg * q   (bf16 2x), one TT per gather call
                prod = worka.tile([P, K * C], bf16)
                for i in range(NCALL):
                    kg_i = kvgs[i][:, :, 2 * b * C : 2 * b * C + C]
                    nc.vector.tensor_tensor(
                        out=_ap(_off(prod[:], i * k_call * C),
                                [[C, k_call], [1, C]]),
                        in0=kg_i, in1=_ap(qt, [[0, k_call], [1, C]]),
                        op=mybir.AluOpType.mult)

                # scores = sum_d prod : 4-level pairwise tree over d
                st1 = small.tile([P, K * HEADS * 8], bf16)
                nc.vector.tensor_tensor(
                    out=_ap(st1[:], [[8, K * HEADS], [1, 8]]),
                    in0=_ap(prod[:], [[d, K * HEADS], [1, 8]]),
                    in1=_ap(_off(prod[:], 8), [[d, K * HEADS], [1, 8]]),
                    op=mybir.AluOpType.add)
                st2 = small.tile([P, K * HEADS * 4], bf16)
                nc.vector.tensor_tensor(
                    out=_ap(st2[:], [[4, K * HEADS], [1, 4]]),
                    in0=_ap(st1[:], [[8, K * HEADS], [1, 4]]),
                    in1=_ap(_off(st1[:], 4), [[8, K * HEADS], [1, 4]]),
                    op=mybir.AluOpType.add)
                st3 = small.tile([P, K * HEADS * 2], f32)
                nc.vector.tensor_tensor(
                    out=_ap(st3[:], [[2, K * HEADS], [1, 2]]),
                    in0=_ap(st2[:], [[4, K * HEADS], [1, 2]]),
                    in1=_ap(_off(st2[:], 2), [[4, K * HEADS], [1, 2]]),
                    op=mybir.AluOpType.add)
                scores = small.tile([P, K * HEADS], f32)       # (k, h)
                nc.vector.tensor_tensor(
                    out=_ap(scores[:], [[1, K * HEADS]]),
                    in0=_ap(st3[:], [[2, K * HEADS]]),
                    in1=_ap(_off(st3[:], 1), [[2, K * HEADS]]),
                    op=mybir.AluOpType.add)

                # expx[(k,h,d)] = exp(scores/4) expanded over d (ScalarE)
                expx = workb.tile([P, K * C], bf16)
                nc.scalar.activation(
                    out=_ap(expx[:], [[C, K], [d, HEADS], [1, d]]),
                    in_=_ap(scores[:], [[HEADS, K], [1, HEADS], [0, d]]),
                    func=mybir.ActivationFunctionType.Exp,
                    scale=1.0 / float(np.sqrt(d)))

                # z[h] = sum_k expx[k,h,0]
                z = small.tile([P, HEADS], f32)
                nc.vector.tensor_reduce(
                    out=z[:],
                    in_=_ap(expx[:], [[d, HEADS], [C, K]]),
                    axis=mybir.AxisListType.X, op=mybir.AluOpType.add)
                rz = small.tile([P, HEADS], f32)
                nc.vector.reciprocal(rz[:], z[:])

                # prod2 = expx * vg  (bf16 2x, both dense)
                prod2 = worka.tile([P, K * C], bf16)
                for i in range(NCALL):
                    vg_i = kvgs[i][:, :, 2 * b * C + C : 2 * (b + 1) * C]
                    nc.vector.tensor_tensor(
                        out=_ap(_off(prod2[:], i * k_call * C),
                                [[C, k_call], [1, C]]),
                        in0=vg_i,
                        in1=_ap(_off(expx[:], i * k_call * C),
                                [[C, k_call], [1, C]]),
                        op=mybir.AluOpType.mult)

                # acc = sum_k prod2 : 5-level pairwise tree over k
                at1 = small.tile([P, K * C // 2], bf16)
                nc.vector.tensor_tensor(
                    out=at1[:], in0=prod2[:, 0 : K * C // 2],
                    in1=prod2[:, K * C // 2 : K * C],
                    op=mybir.AluOpType.add)
                at2 = small.tile([P, K * C // 4], bf16)
                nc.vector.tensor_tensor(
                    out=at2[:], in0=at1[:, 0 : K * C // 4],
                    in1=at1[:, K * C // 4 : K * C // 2],
                    op=mybir.AluOpType.add)
                at3 = small.tile([P, K * C // 8], bf16)
                nc.vector.tensor_tensor(
                    out=at3[:], in0=at2[:, 0 : K * C // 8],
                    in1=at2[:, K * C // 8 : K * C // 4],
                    op=mybir.AluOpType.add)
                at4 = small.tile([P, 2 * C], bf16)
                nc.vector.tensor_tensor(
                    out=at4[:], in0=at3[:, 0 : 2 * C], in1=at3[:, 2 * C : 4 * C],
                    op=mybir.AluOpType.add)
                acc = small.tile([P, C], f32)
                nc.vector.tensor_tensor(
                    out=acc[:], in0=at4[:, 0:C], in1=at4[:, C : 2 * C],
                    op=mybir.AluOpType.add)

                # out = acc * (1/z) + bv   (bv exact: sum_k attn == 1)
                sc = small.tile([P, C], f32)
                nc.vector.tensor_tensor(
                    out=sc[:], in0=acc[:],
                    in1=_ap(rz[:], [[1, HEADS], [0, d]]),
                    op=mybir.AluOpType.mult)
                outt = small.tile([P, C], f32)
                nc.vector.tensor_tensor(
                    out=outt[:], in0=sc[:], in1=bvrep_sb[:],
                    op=mybir.AluOpType.add)
                nc.sync.dma_start(
                    out=outp[b * cfg.N_own + t * P : b * cfg.N_own + (t + 1) * P, :],
                    in_=outt[:])

    nc.compile()
    return nc


def make_in_maps(cfg: Cfg, x, Wq, bq, Wk, bk, Wv, bv, neighbor_index):
    import ml_dtypes

    N, K, C, B = cfg.N, cfg.K, cfg.C, cfg.B
    T_own, N_own = cfg.n_own_tiles, cfg.N_own
    bf16 = ml_dtypes.bfloat16

    x = np.asarray(x, np.float32)
    wqkv = np.ascontiguousarray(np.concatenate(
        [np.asarray(Wq, np.float32).T, np.asarray(Wk, np.float32).T,
         np.asarray(Wv, np.float32).T], axis=1)).astype(bf16)
    brow = np.zeros((C, 3 * C), np.float32)
    brow[0, :] = np.concatenate(
        [np.asarray(bq, np.float32), np.asarray(bk, np.float32),
         np.asarray(bv, np.float32)])
    nbr = np.asarray(neighbor_index, np.int64)
    xtb = np.ascontiguousarray(x.transpose(0, 2, 1))   # [B, C, N]

    in_maps = []
    for c in range(cfg.n_cores):
        own = np.arange(c * N_own, (c + 1) * N_own)
        rest = np.concatenate(
            [np.arange(0, c * N_own), np.arange((c + 1) * N_own, N)])
        perm = np.concatenate([own, rest])
        inv = np.empty(N, np.int64)
        inv[perm] = np.arange(N)

        xt_c = np.ascontiguousarray(
            xtb[:, :, perm].transpose(1, 0, 2).reshape(C, B * N)).astype(bf16)

        nb = inv[nbr[own]]                                  # [N_own, K]
        vals = nb.reshape(T_own, P, K).transpose(0, 2, 1)   # [T, k, nl]
        vals = vals.reshape(T_own, NCALL, (K // NCALL) * P)
        a = vals.reshape(T_own, NCALL, (K // NCALL) * P // 16, 16)
        a = a.transpose(3, 0, 1, 2)                          # [16, T, NCALL, S]
        rep = np.tile(a, (8, 1, 1, 1))                       # [128, ...]
        idxw = np.ascontiguousarray(
            rep.reshape(P, T_own * (K * P // 16)).astype(np.int16))

        in_maps.append({
            "xt": xt_c, "wqkv": wqkv, "brow": brow, "idxw": idxw,
        })
    return in_maps


_CACHE = {}


def _get_nc(cfg: Cfg):
    key = (cfg.N, cfg.K, cfg.C, cfg.n_cores, cfg.B)
    if key not in _CACHE:
        _CACHE[key] = build_nc(cfg)
    return _CACHE[key]


def kernel(x, Wq, bq, Wk, bk, Wv, bv, neighbor_index, _trace=False):
    from concourse.bass_utils import run_bass_kernel_spmd

    x = np.asarray(x)
    B, N, C = x.shape
    K = np.asarray(neighbor_index).shape[1]
    cfg = Cfg(N=N, K=K, C=C, n_cores=8, B=B)
    nc = _get_nc(cfg)
    in_maps = make_in_maps(cfg, x, Wq, bq, Wk, bk, Wv, bv, neighbor_index)
    res = run_bass_kernel_spmd(nc, in_maps, core_ids=list(range(cfg.n_cores)),
                               trace=_trace)
    out = np.empty((B, N, C), np.float32)
    for c in range(cfg.n_cores):
        o = res.results[c]["out"].reshape(B, cfg.N_own, C)
        out[:, c * cfg.N_own : (c + 1) * cfg.N_own, :] = o
    if _trace:
        kernel.last_results = res
    return out


# revision 10
# speedup vs baseline: 1.6464x; 1.1688x over previous
"""Local softmax attention (GNN message passing) on 8 Trainium2 NeuronCores.

Math (per batch b, node n):
  q/k/v = x @ W{q,k,v}.T + b{q,k,v}              [N, 128], 8 heads x d=16
  scores[n,k,h] = sum_d q[n,h,d] * k[nbr(n,k),h,d] / sqrt(d)
  attn = softmax over k (32 neighbors)
  out[n,h,d] = sum_k attn[n,k,h] * v[nbr(n,k),h,d]

Sharding: 8 cores, each owning a 2048-node range (both batches).

The dominant cost on TRN2 is the neighbor gather: SWDGE descriptor
generation costs ~8.6 ns per gathered row (Q7 software), so k|v for BOTH
batches are packed into one 1KB DRAM row per node and each gathered row
serves both batch instances -> 65536 rows per core.  dma_gather is
limited to ~1024 indices per call (the 128-deep SWDGE descriptor ring:
2048 idxs hangs the exec unit) so each 128-node tile's 4096-row gather is
split into 4 calls.

Phase 1 (projections): every core redundantly computes k|v (bf16) for
all nodes of both batches on the TensorEngine (x^T tiles stationary,
all-bf16 matmul) and writes the packed rows to a private DRAM scratch in
1 MiB group transfers (8 tiles x both batches per store); q (bf16) for
its own nodes stays in SBUF with the bq bias added on VectorE.  bk
cancels in the softmax; bv is added exactly at the end (sum_k attn == 1).

Phase 2 (per 128-node tile, per batch): VectorE does q*kg (bf16 2x),
a 4-level pairwise-add tree over d for the scores (last levels fp32),
softmax denominator, attn*vg (bf16 2x, exp expanded over d by ScalarE so
both operands are dense), a 5-level pairwise tree over k, the 1/Z
scale and the bv add.  ScalarE does the exp (reading scores with a
step-0 broadcast AP so the output is already expanded over d).

SPMD: all 8 cores run the identical program; per-core variation is data
only (each core's x^T is permuted so its own 2048 nodes come first, and
gather indices are remapped into that row space).
"""

import os
import sys

sys.path.insert(0, "/opt/trn_rl_repo")

from contextlib import ExitStack

import numpy as np

import concourse.bacc as bacc
import concourse.bass as bass
import concourse.tile as tile
from concourse import mybir

HEADS = 8
P = 128
NCALL = 4          # gather calls per tile (1024 idxs each)
GT = 8             # phase-1 tiles per DMA group


class Cfg:
    def __init__(self, N=16384, K=32, C=128, n_cores=8, B=2):
        self.N, self.K, self.C, self.n_cores, self.B = N, K, C, n_cores, B
        self.N_own = N // n_cores
        self.n_all_tiles = N // P
        self.n_own_tiles = self.N_own // P
        self.d = C // HEADS


def _ap(base, dims):
    return bass.AP(tensor=base.tensor, offset=base.offset,
                   ap=[base.ap[0]] + [list(x) for x in dims])


def _off(base, elems):
    return bass.AP(tensor=base.tensor, offset=base.offset + elems,
                   ap=base.ap)


def build_nc(cfg: Cfg):
    N, K, C, B = cfg.N, cfg.K, cfg.C, cfg.B
    H3 = 3 * C
    R = 2 * B * C              # packed row elems (k|v per batch): 512
    f32, bf16, i16 = mybir.dt.float32, mybir.dt.bfloat16, mybir.dt.int16
    T_all, T_own = cfg.n_all_tiles, cfg.n_own_tiles
    d = cfg.d
    ni_call = K * P // NCALL
    k_call = K // NCALL
    cols_call = ni_call // 16
    idx_cols = NCALL * cols_call
    n_grp = T_all // GT

    nc = bacc.Bacc("TRN2", target_bir_lowering=False, debug=False)

    xt = nc.dram_tensor("xt", [C, B * N], bf16, kind="ExternalInput")
    wqkv = nc.dram_tensor("wqkv", [C, H3], bf16, kind="ExternalInput")
    brow = nc.dram_tensor("brow", [C, H3], f32, kind="ExternalInput")
    idxw = nc.dram_tensor("idxw", [P, T_own * idx_cols], i16,
                          kind="ExternalInput")
    outp = nc.dram_tensor("out", [B * cfg.N_own, C], f32,
                          kind="ExternalOutput")

    with tile.TileContext(nc) as tc, ExitStack() as ctx:
        const = ctx.enter_context(tc.tile_pool(name="const", bufs=1))
        xload = ctx.enter_context(tc.tile_pool(name="xload", bufs=2))
        psum = ctx.enter_context(tc.tile_pool(name="psum", bufs=4, space="PSUM"))
        kvgrp = ctx.enter_context(tc.tile_pool(name="kvgrp", bufs=2))
        dram = ctx.enter_context(tc.tile_pool(name="dram", bufs=1, space="DRAM"))
        gath = ctx.enter_context(tc.tile_pool(name="gath", bufs=3 * NCALL))
        worka = ctx.enter_context(tc.tile_pool(name="worka", bufs=2))
        workb = ctx.enter_context(tc.tile_pool(name="workb", bufs=1))
        # DVE-internal scratch: DVE is in-order, so same-engine WAW/WAR
        # needs no double buffering.  Cross-engine tiles (scores -> ScalarE,
        # outt -> DMA) live in smx with 2 bufs.
        small = ctx.enter_context(tc.tile_pool(name="small", bufs=1))
        smx = ctx.enter_context(tc.tile_pool(name="smx", bufs=2))

        # --- constants ---
        wqkv_sb = const.tile([C, H3], bf16)
        nc.sync.dma_start(out=wqkv_sb[:], in_=wqkv[:, :])
        bvrep_sb = const.tile([P, C], f32)   # bv replicated across partitions
        b0 = brow[0:1, 2 * C : 3 * C]
        nc.gpsimd.dma_start(
            out=bvrep_sb[:],
            in_=bass.AP(tensor=b0.tensor, offset=b0.offset, ap=[[0, P], [1, C]]))
        bqrep_sb = const.tile([P, C], f32)   # bq replicated across partitions
        q0 = brow[0:1, 0:C]
        nc.gpsimd.dma_start(
            out=bqrep_sb[:],
            in_=bass.AP(tensor=q0.tensor, offset=q0.offset, ap=[[0, P], [1, C]]))
        idx_sb = const.tile([P, T_own * idx_cols], i16)
        nc.sync.dma_start(out=idx_sb[:], in_=idxw[:, :])
        q_all = const.tile([P, T_own * B * C], bf16)   # [t][b][hd]

        kv_dram = dram.tile([N, R], bf16)

        # --- phase 1: projections (both batches, all nodes) ---
        # Grouped DMA: load 8 tiles of x^T per batch in one 256 KiB
        # transfer, store 8 tiles' packed k|v rows (both batches) in one
        # 1 MiB transfer.
        for g in range(n_grp):
            xt_g = xload.tile([P, B, GT, P], bf16)
            for b in range(B):
                nc.sync.dma_start(
                    out=xt_g[:, b],
                    in_=xt[:, b * N + g * GT * P : b * N + (g + 1) * GT * P])
            kv_g = kvgrp.tile([P, GT, B, 2 * C], bf16)
            for t8 in range(GT):
                t = g * GT + t8
                for b in range(B):
                    pt = psum.tile([P, H3], f32)
                    if t < T_own:
                        nc.tensor.matmul(out=pt[:], lhsT=xt_g[:, b, t8],
                                         rhs=wqkv_sb[:],
                                         start=True, stop=True)
                        q_slot = (t * B + b) * C
                        nc.vector.tensor_tensor(
                            out=q_all[:, q_slot : q_slot + C], in0=pt[:, 0:C],
                            in1=bqrep_sb[:], op=mybir.AluOpType.add)
                    else:
                        nc.tensor.matmul(out=pt[:, C:H3],
                                         lhsT=xt_g[:, b, t8],
                                         rhs=wqkv_sb[:, C:H3],
                                         start=True, stop=True)
                    # split PSUM->SBUF copies between ScalarE and VectorE
                    if b == 0:
                        nc.scalar.activation(
                            out=kv_g[:, t8, b], in_=pt[:, C:H3],
                            func=mybir.ActivationFunctionType.Copy)
                    else:
                        nc.vector.tensor_scalar_add(kv_g[:, t8, b],
                                                    pt[:, C:H3], 0.0)
            # rows g*1024 + t8*128 + p, all 512 cols
            dst = kv_dram[g * GT * P : (g + 1) * GT * P, :]
            nc.sync.dma_start(
                out=bass.AP(tensor=dst.tensor, offset=dst.offset,
                            ap=[[R, P], [R * P, GT], [1, R]]),
                in_=kv_g[:])

        # --- phase 2: gather + attention ---
        # One register for all gather calls (a per-call MOVE creates WAR
        # hazards that stall the Pool queue); one SBUF sub-tile per call so
        # consecutive gathers don't WAW-serialize on transfer completion.
        ni_reg = nc.gpsimd.to_reg(ni_call)
        for t in range(T_own):
            kvgs = []
            for i in range(NCALL):
                kvg_i = gath.tile([P, k_call, R], bf16)
                c0 = t * idx_cols + i * cols_call
                nc.gpsimd.dma_gather(
                    out_ap=kvg_i[:],
                    in_ap=kv_dram[:],
                    idxs_ap=idx_sb[:, c0 : c0 + cols_call],
                    num_idxs=ni_call,
                    num_idxs_reg=ni_reg,
                    elem_size=R,
                )
                kvgs.append(kvg_i)

            for b in range(B):
                qt = q_all[:, (t * B + b) * C : (t * B + b + 1) * C]

                # prod[(k,h,d)] = kg * q   (bf16 2x), one TT per gather call
                prod = worka.tile([P, K * C], bf16)
                for i in range(NCALL):
                    kg_i = kvgs[i][:, :, 2 * b * C : 2 * b * C + C]
                    nc.vector.tensor_tensor(
                        out=_ap(_off(prod[:], i * k_call * C),
                                [[C, k_call], [1, C]]),
                        in0=kg_i, in1=_ap(qt, [[0, k_call], [1, C]]),
                        op=mybir.AluOpType.mult)

                # scores = sum_d prod : 4-level pairwise tree over d
                st1 = small.tile([P, K * HEADS * 8], bf16)
                nc.vector.tensor_tensor(
                    out=_ap(st1[:], [[8, K * HEADS], [1, 8]]),
                    in0=_ap(prod[:], [[d, K * HEADS], [1, 8]]),
                    in1=_ap(_off(prod[:], 8), [[d, K * HEADS], [1, 8]]),
                    op=mybir.AluOpType.add)
                st2 = small.tile([P, K * HEADS * 4], bf16)
                nc.vector.tensor_tensor(
                    out=_ap(st2[:], [[4, K * HEADS], [1, 4]]),
                    in0=_ap(st1[:], [[8, K * HEADS], [1, 4]]),
                    in1=_ap(_off(st1[:], 4), [[8, K * HEADS], [1, 4]]),
                    op=mybir.AluOpType.add)
                st3 = small.tile([P, K * HEADS * 2], f32)
                nc.vector.tensor_tensor(
                    out=_ap(st3[:], [[2, K * HEADS], [1, 2]]),
                    in0=_ap(st2[:], [[4, K * HEADS], [1, 2]]),
                    in1=_ap(_off(st2[:], 2), [[4, K * HEADS], [1, 2]]),
                    op=mybir.AluOpType.add)
                scores = smx.tile([P, K * HEADS], f32)       # (k, h)
                nc.vector.tensor_tensor(
                    out=_ap(scores[:], [[1, K * HEADS]]),
                    in0=_ap(st3[:], [[2, K * HEADS]]),
                    in1=_ap(_off(st3[:], 1), [[2, K * HEADS]]),
                    op=mybir.AluOpType.add)

                # expx[(k,h,d)] = exp(scores/4) expanded over d (ScalarE)
                expx = workb.tile([P, K * C], bf16)
                nc.scalar.activation(
                    out=_ap(expx[:], [[C, K], [d, HEADS], [1, d]]),
                    in_=_ap(scores[:], [[HEADS, K], [1, HEADS], [0, d]]),
                    func=mybir.ActivationFunctionType.Exp,
                    scale=1.0 / float(np.sqrt(d)))

                # z[h] = sum_k expx[k,h,0]
                z = small.tile([P, HEADS], f32)
                nc.vector.tensor_reduce(
                    out=z[:],
                    in_=_ap(expx[:], [[d, HEADS], [C, K]]),
                    axis=mybir.AxisListType.X, op=mybir.AluOpType.add)
                rz = small.tile([P, HEADS], f32)
                nc.vector.reciprocal(rz[:], z[:])

                # prod2 = expx * vg  (bf16 2x, both dense)
                prod2 = worka.tile([P, K * C], bf16)
                for i in range(NCALL):
                    vg_i = kvgs[i][:, :, 2 * b * C + C : 2 * (b + 1) * C]
                    nc.vector.tensor_tensor(
                        out=_ap(_off(prod2[:], i * k_call * C),
                                [[C, k_call], [1, C]]),
                        in0=vg_i,
                        in1=_ap(_off(expx[:], i * k_call * C),
                                [[C, k_call], [1, C]]),
                        op=mybir.AluOpType.mult)

                # acc = sum_k prod2 : 5-level pairwise tree over k
                at1 = small.tile([P, K * C // 2], bf16)
                nc.vector.tensor_tensor(
                    out=at1[:], in0=prod2[:, 0 : K * C // 2],
                    in1=prod2[:, K * C // 2 : K * C],
                    op=mybir.AluOpType.add)
                at2 = small.tile([P, K * C // 4], bf16)
                nc.vector.tensor_tensor(
                    out=at2[:], in0=at1[:, 0 : K * C // 4],
                    in1=at1[:, K * C // 4 : K * C // 2],
                    op=mybir.AluOpType.add)
                at3 = small.tile([P, K * C // 8], bf16)
                nc.vector.tensor_tensor(
                    out=at3[:], in0=at2[:, 0 : K * C // 8],
                    in1=at2[:, K * C // 8 : K * C // 4],
                    op=mybir.AluOpType.add)
                at4 = small.tile([P, 2 * C], bf16)
                nc.vector.tensor_tensor(
                    out=at4[:], in0=at3[:, 0 : 2 * C], in1=at3[:, 2 * C : 4 * C],
                    op=mybir.AluOpType.add)
                acc = small.tile([P, C], f32)
                nc.vector.tensor_tensor(
                    out=acc[:], in0=at4[:, 0:C], in1=at4[:, C : 2 * C],
                    op=mybir.AluOpType.add)

                # out = acc * (1/z) + bv   (bv exact: sum_k attn == 1)
                sc = small.tile([P, C], f32)
                nc.vector.tensor_tensor(
                    out=sc[:], in0=acc[:],
                    in1=_ap(rz[:], [[1, HEADS], [0, d]]),
                    op=mybir.AluOpType.mult)
                outt = smx.tile([P, C], f32)
                nc.vector.tensor_tensor(
                    out=outt[:], in0=sc[:], in1=bvrep_sb[:],
                    op=mybir.AluOpType.add)
                nc.sync.dma_start(
                    out=outp[b * cfg.N_own + t * P : b * cfg.N_own + (t + 1) * P, :],
                    in_=outt[:])

    nc.compile()
    return nc


def make_in_maps(cfg: Cfg, x, Wq, bq, Wk, bk, Wv, bv, neighbor_index):
    import ml_dtypes

    N, K, C, B = cfg.N, cfg.K, cfg.C, cfg.B
    T_own, N_own = cfg.n_own_tiles, cfg.N_own
    bf16 = ml_dtypes.bfloat16

    x = np.asarray(x, np.float32)
    wqkv = np.ascontiguousarray(np.concatenate(
        [np.asarray(Wq, np.float32).T, np.asarray(Wk, np.float32).T,
         np.asarray(Wv, np.float32).T], axis=1)).astype(bf16)
    brow = np.zeros((C, 3 * C), np.float32)
    brow[0, :] = np.concatenate(
        [np.asarray(bq, np.float32), np.asarray(bk, np.float32),
         np.asarray(bv, np.float32)])
    nbr = np.asarray(neighbor_index, np.int64)
    xtb = np.ascontiguousarray(x.transpose(0, 2, 1))   # [B, C, N]

    in_maps = []
    for c in range(cfg.n_cores):
        own = np.arange(c * N_own, (c + 1) * N_own)
        rest = np.concatenate(
            [np.arange(0, c * N_own), np.arange((c + 1) * N_own, N)])
        perm = np.concatenate([own, rest])
        inv = np.empty(N, np.int64)
        inv[perm] = np.arange(N)

        xt_c = np.ascontiguousarray(
            xtb[:, :, perm].transpose(1, 0, 2).reshape(C, B * N)).astype(bf16)

        nb = inv[nbr[own]]                                  # [N_own, K]
        vals = nb.reshape(T_own, P, K).transpose(0, 2, 1)   # [T, k, nl]
        vals = vals.reshape(T_own, NCALL, (K // NCALL) * P)
        a = vals.reshape(T_own, NCALL, (K // NCALL) * P // 16, 16)
        a = a.transpose(3, 0, 1, 2)                          # [16, T, NCALL, S]
        rep = np.tile(a, (8, 1, 1, 1))                       # [128, ...]
        idxw = np.ascontiguousarray(
            rep.reshape(P, T_own * (K * P // 16)).astype(np.int16))

        in_maps.append({
            "xt": xt_c, "wqkv": wqkv, "brow": brow, "idxw": idxw,
        })
    return in_maps


_CACHE = {}


def _get_nc(cfg: Cfg):
    key = (cfg.N, cfg.K, cfg.C, cfg.n_cores, cfg.B)
    if key not in _CACHE:
        _CACHE[key] = build_nc(cfg)
    return _CACHE[key]


def kernel(x, Wq, bq, Wk, bk, Wv, bv, neighbor_index, _trace=False):
    from concourse.bass_utils import run_bass_kernel_spmd

    x = np.asarray(x)
    B, N, C = x.shape
    K = np.asarray(neighbor_index).shape[1]
    cfg = Cfg(N=N, K=K, C=C, n_cores=8, B=B)
    nc = _get_nc(cfg)
    in_maps = make_in_maps(cfg, x, Wq, bq, Wk, bk, Wv, bv, neighbor_index)
    res = run_bass_kernel_spmd(nc, in_maps, core_ids=list(range(cfg.n_cores)),
                               trace=_trace)
    out = np.empty((B, N, C), np.float32)
    for c in range(cfg.n_cores):
        o = res.results[c]["out"].reshape(B, cfg.N_own, C)
        out[:, c * cfg.N_own : (c + 1) * cfg.N_own, :] = o
    if _trace:
        kernel.last_results = res
    return out
